# revision 17
# baseline (speedup 1.0000x reference)
"""Dimension-adaptive max pooling for sensors — Trainium2 Bass kernel.

Problem: x (64, 512, 48, 64) f32 -> out (64, 16*6*64) = (64, 6144) f32.
Adaptive max pool over spatial dims (512, 48) into (16, 6) bins. Since
512/16 = 32 and 48/6 = 8 exactly, each output bin is a plain max over a
(32, 8) window:

  out[b, iw*384 + ih*64 + m] = max_{r<32, hh<8} x[b, iw*32+r, ih*8+hh, m]

Sharding: pure data parallel over batch. 8 cores x 8 samples each.

Per-core layout: x[b] is a contiguous (512, 48*64) block and the 16
w-bins tile it exactly, so the per-core input is a flat (128, 98304)
array where partition p = (b_local*16 + iw) owns one contiguous w-bin
(32 rows x 3072 floats). The per-partition reduction keeps (ih=6, m=64)
-> 384 outputs = exactly the per-(b, iw) slice of the output. Both
input and output DMAs are perfectly coalesced, no transposes.

Final pipeline (per core), evolved over several traced iterations:
 - 17 sub-loads on the single Pool SWDGE queue (nc.gpsimd): 15 x 2
   w-rows (3.15 MB) + 1-row tiles for rows 30 and 31, into 6 rotating
   SBUF sub-slots. SWDGE increments the completion sem +1 per SDMA
   engine after that engine's own portion, so sem thresholds bound
   every engine's progress (HWDGE's +16 does not, and was observed to
   let the DVE read stale slots). After each load, a tiny same-queue
   readback DMA re-reads the slot tail; the DVE gates each tile on the
   readback sem AND on the NEXT load's completion sem (~7 us margin) —
   semaphore updates can lead SDMA write retirement under heavy NoC
   load (NTFF profiling), so both belts are worn.
 - DVE folds w-rows with unit-stride tensor_tensor max chains into TWO
   alternating accumulators: a single in-place chain ran at 1.26
   cyc/elem (dependent-op stall); alternating restores ~1.05. The
   strided 4D tensor_reduce alternative runs at only 1.61 cyc/elem.
 - h-fold (8 -> 1 cols) as pairwise TT-max trees. Row 31 (the last
   load) bypasses the accumulators via its own fold path, and the
   accumulator tree folds while waiting for it, so the critical chain
   after the last byte lands is ~4 us of DVE work + the output DMA.
 - Output via the ACT HWDGE ring (nc.scalar) so it never queues behind
   loads. DVE total ~105 us < DMA stream ~147 us (SWDGE sustains
   ~21.5 GB/s x 16 SDMA engines vs the 358 GB/s/core HBM limit).
Raw Bass (not Tile): slot-reuse ordering lives in standalone sequencer
wait_ge instructions; Tile attaches 2 waits to the DMA instruction
itself, which overflows DMA_DIRECT2D's 1-wait budget in walrus codegen.
"""

import sys

sys.path.insert(0, "/opt/trn_rl_repo")

import numpy as np

import concourse.bass as bass
from concourse import mybir
from concourse.bass_utils import run_bass_kernel_spmd

N_CORES = 8
B, W, H, M = 64, 512, 48, 64
POOL_W, POOL_H = 16, 6
BIN_W, BIN_H = W // POOL_W, H // POOL_H  # 32, 8
B_LOC = B // N_CORES  # 8 samples per core
P = B_LOC * POOL_W  # 128 partitions = (b_local, iw)
ROW = H * M  # 3072 floats per w-row per partition
FREE = BIN_W * ROW  # 98304 elems per partition (one w-bin)
OUT_FREE = POOL_H * M  # 384
N_SLOTS = 6
SLOT_ROWS = 2
# rows 0..29 in 2-row tiles feeding the accumulators, then row 30
# (last accumulator update), then row 31 which bypasses the
# accumulators entirely via its own fold path — so the critical chain
# after the last byte lands is just fold8+fold4+fold2+max, ~4 us.
TILES = [(k * 2, 2) for k in range(15)] + [(30, 1), (31, 1)]
NT = len(TILES)  # 17

F32 = mybir.dt.float32

_cached = {}


def _build():
    if "nc" in _cached:
        return _cached["nc"]
    nc = bass.Bass()
    x = nc.dram_tensor("x", [P, FREE], F32, kind="ExternalInput")
    out = nc.dram_tensor("out", [P, OUT_FREE], F32, kind="ExternalOutput")

    with (
        nc.sbuf_tensor([P, N_SLOTS, SLOT_ROWS * ROW], F32) as slots,
        nc.sbuf_tensor([P, 16], F32) as scratch,
        nc.sbuf_tensor([P, ROW], F32) as acc_a,
        nc.sbuf_tensor([P, ROW], F32) as acc_b,
        nc.sbuf_tensor([P, POOL_H * 4 * M], F32) as fa,
        nc.sbuf_tensor([P, POOL_H * 4 * M], F32) as fb,
        nc.sbuf_tensor([P, POOL_H * 2 * M], F32) as tmp2,
        nc.sbuf_tensor([P, OUT_FREE], F32) as res,
        nc.semaphore() as dma_sem,  # load completions, +16 each
        nc.semaphore() as rb_sem,  # readback completions: data readable
        nc.semaphore() as free_sem,  # DVE done consuming tile j, +1
        nc.semaphore() as res_sem,  # final result ready
        nc.semaphore() as out_sem,  # output DMA completion
        nc.Block() as block,
    ):

        @block.gpsimd
        def _(g):
            # SWDGE, not HWDGE: per-engine FIFO ordering is the backbone of
            # the completion story. After every load, a tiny readback DMA on
            # the SAME queue reads the just-written slot tail: each engine's
            # readback descriptors sit behind its data descriptors in the
            # ring, and the SBUF port serializes the read behind the posted
            # writes — so the readback's per-engine +1 increments prove the
            # tile's data is actually readable (a plain load-completion sem
            # was observed to lead the data under NTFF profiling load).
            for k, (row0, nrows) in enumerate(TILES):
                if k >= N_SLOTS:
                    g.wait_ge(free_sem, k - N_SLOTS + 1)
                g.dma_start(
                    out=slots[:, k % N_SLOTS, 0 : nrows * ROW],
                    in_=x[:, row0 * ROW : (row0 + nrows) * ROW],
                ).then_inc(dma_sem, 16)
                g.dma_start(
                    out=scratch[:, :],
                    in_=slots[:, k % N_SLOTS, nrows * ROW - 16 : nrows * ROW],
                ).then_inc(rb_sem, 16)

        @block.scalar
        def _(sc):
            sc.wait_ge(res_sem, 1)
            sc.dma_start(out=out[:, :], in_=res[:, :]).then_inc(out_sem, 16)
            sc.wait_ge(out_sem, 16)

        @block.vector
        def _(v):
            mx = mybir.AluOpType.max

            def row(sl, r):
                return sl[:, r * ROW : (r + 1) * ROW]

            def fold(dst, src, hh, src_is_ap=False):
                a = (src if src_is_ap else src[:, :]).rearrange(
                    "p (ih hh m) -> p ih hh m", ih=POOL_H, hh=hh, m=M
                )
                return v.tensor_tensor(
                    out=dst[:, :],
                    in0=a[:, :, 0 : hh // 2, :],
                    in1=a[:, :, hh // 2 : hh, :],
                    op=mx,
                )

            for k, (row0, nrows) in enumerate(TILES[:-1]):
                # tile k readable once its readback completed AND load k+1
                # completed (a full ~7 us margin on top — write retirement
                # can lag semaphore updates under heavy NoC load)
                v.wait_ge(rb_sem, 16 * (k + 1))
                v.wait_ge(dma_sem, 16 * (k + 2))
                sl = slots[:, k % N_SLOTS, :]
                if k == 0:
                    ins = v.tensor_tensor(
                        out=acc_a[:, :], in0=row(sl, 0), in1=row(sl, 1), op=mx
                    )
                elif k == 1:
                    ins = v.tensor_tensor(
                        out=acc_b[:, :], in0=row(sl, 0), in1=row(sl, 1), op=mx
                    )
                else:
                    for r in range(nrows):
                        acc = acc_a if ((row0 + r) % 2 == 0) else acc_b
                        ins = v.tensor_tensor(
                            out=acc[:, :], in0=acc[:, :], in1=row(sl, r), op=mx
                        )
                ins.then_inc(free_sem, 1)
                if k == NT - 3:
                    # acc_b's final update was row 29 (tile NT-3); fold it
                    # while waiting for the row-30 load
                    fold(fb, acc_b, BIN_H)

            # acc_a complete (row 30 was tile NT-2): fold the accumulator
            # tree down to 384 while the row-31 load + chaser complete
            fold(fa, acc_a, BIN_H)
            v.tensor_tensor(out=fa[:, :], in0=fa[:, :], in1=fb[:, :], op=mx)
            fold(tmp2, fa, 4)
            fold(res, tmp2, 2)

            # row 31 bypasses the accumulators: fold it directly and merge.
            # Its readback (issued after it on the same ring) doubles as the
            # completion margin for the final load.
            k = NT - 1
            v.wait_ge(rb_sem, 16 * (k + 1))
            sl = slots[:, k % N_SLOTS, :]
            fold(fb, sl[:, 0:ROW], BIN_H, src_is_ap=True)
            fold(tmp2, fb, 4)
            fold(fb[:, 0:OUT_FREE], tmp2, 2)
            v.tensor_tensor(
                out=res[:, :], in0=res[:, :], in1=fb[:, 0:OUT_FREE], op=mx
            ).then_inc(res_sem, 1)

    _cached["nc"] = nc
    return nc


def kernel(x: np.ndarray, **run_kwargs) -> np.ndarray:
    nc = _build()
    x = np.ascontiguousarray(x, dtype=np.float32)
    xs = x.reshape(N_CORES, P, FREE)
    in_maps = [{"x": xs[c]} for c in range(N_CORES)]
    r = run_bass_kernel_spmd(nc, in_maps, core_ids=list(range(N_CORES)), **run_kwargs)
    out = np.concatenate(
        [r.results[c]["out"].reshape(B_LOC, POOL_W * OUT_FREE) for c in range(N_CORES)],
        axis=0,
    )
    if run_kwargs:
        return out, r
    return out
